# revision 17
# baseline (speedup 1.0000x reference)
"""Trainium2 Bass kernel for nn_ComplexMixture.

Per batch element b (R = input_real[b] [S,D], I = input_imag[b] [S,D], w [S]):
    out_r = (w*R)^T R + (w*I)^T I        (symmetric)
    out_i = (w*I)^T R - (w*R)^T I        (antisymmetric)

With A = sqrt(w)*R, B = sqrt(w)*I (w >= 0):
    N  = B^T A            (full 768x768, 48 matmuls)
    G  = (A+B)^T (A+B)    (upper 9 of 12 [128,384] pair-tiles, 36 matmuls)
    out_r = G - N - N^T   (symmetric   -> upper tiles only)
    out_i = N - N^T       (antisymmetric -> upper tiles only)

vs the 4-term scheme (144 matmuls = 55.3k PE cycles) this needs 44.9k PE
cycles, and keeps single-pass PSUM evacuation: every out_r combine happens
ON THE PE inside the psG accumulation group
    psg = G(4 mm) - N(1 mm: lhsT=-I, rhs=pN) - N^T(1 mm: lhsT=-I, rhs=nt)
where pN = fp16 SBUF copy of N and nt = fp16 SBUF copy of psnt = N^T
(3 fp16 PE transposes of pN, ~56ns each).  The transposes lead each pair
so the nt evacuation hides under that same pair's G matmuls; no
cross-pair dependency chains and a ~1.3us closing tail.  Evac:
out_r = one DVE cast, out_i = one DVE scalar_tensor_tensor (-psnt + pN),
oin (= -out_i, host's antisymmetric mirror) = 2-matmul mini-bank + copy.

Timing structure (measured): exec ~= end-of-last-work + ~4.2us fixed exit
(drain + tile barrier + NEFF semaphore sweep).  The binding constraint on
the front is DMA intake: ~1.57MB of fp16 input over three rings at
~90/70/145 GB/s; the N phase consumes a k-chunk every ~2us, so chunks are
loaded in [P,384] half-units round-robined across rings in k order
(r halves on sync+scalar HWDGE, i halves on the faster gpsimd SWDGE) to
keep arrival cadence ahead of PE demand.  Dummy matmuls bridge the PE
from the preamble into the stream (p-state ramp).

Outputs are stored fp16 (host upcasts: pure dtype cast).  out_r's
strictly-lower [384,384] block and out_i's (via the device-negated oin
tile) are host-mirrored by pure transpose copies.  Sharding:
data-parallel over batch, one element per NeuronCore (B == 8 == n_cores).
"""

import sys
import types

import numpy as np

# If the environment requests tracing (BASS_TRACE=1) but the image lacks
# antenv.axon_hooks, bass_utils would crash importing it; provide a no-op
# hook registry so tracing degrades gracefully instead.
try:
    import antenv.axon_hooks  # noqa: F401
except ImportError:
    _hooks = types.ModuleType("antenv.axon_hooks")
    _hooks._hook = None
    _hooks.set_axon_ntff_profile_hook = lambda h: setattr(_hooks, "_hook", h)
    _hooks.get_axon_ntff_profile_hook = lambda: _hooks._hook
    sys.modules["antenv.axon_hooks"] = _hooks

import concourse.bacc as bacc
import concourse.bass_utils as bass_utils
import concourse.mybir as mybir
import concourse.tile as tile

B, S, D = 8, 512, 768
P = 128          # SBUF/PSUM partitions; matmul contraction tile
KC = S // P      # 4 contraction chunks per operand
MT = D // P      # 6 output row tiles
NW = 384         # matmul moving free dim (<=512 fp32 PSUM bank)
NB = D // NW     # 2 output column blocks
HD = D // 2      # half row width (intake half-units)
N_CORES = 8
N_PREWARM = 9   # dummy 512-col matmuls bridging preamble -> first real mm;
                 # ~3.7us of continuous PE activity so the HAM clock grant
                 # (full 2.4GHz ~6us after sustained activity starts) lands
                 # as early as possible

# 9 computed (m, n) pair-tiles: full upper at [384] granularity; n=0 pairs
# first (their transposes only need pN rows 0..2 from the first N group)
PAIRS = [(0, 0), (1, 0), (2, 0), (0, 1), (1, 1), (2, 1), (3, 1), (4, 1), (5, 1)]

_CACHE: dict = {}


def _build():
    f32, f16 = mybir.dt.float32, mybir.dt.float16
    add_op = mybir.AluOpType.add
    mul_op = mybir.AluOpType.mult
    nc = bacc.Bacc(
        "TRN2", target_bir_lowering=False, debug=False, num_devices=N_CORES
    )
    # Host-packed partition-major: r_in[p, k*D:(k+1)*D] = R[k*P+p, :].
    r_d = nc.dram_tensor("r_in", [P, KC * D], f16, kind="ExternalInput").ap()
    i_d = nc.dram_tensor("i_in", [P, KC * D], f16, kind="ExternalInput").ap()
    s_d = nc.dram_tensor("s_in", [P, KC], f32, kind="ExternalInput").ap()
    # [+I | -I | -2I] fp16
    e_d = nc.dram_tensor("e_in", [P, 3 * P], f16, kind="ExternalInput").ap()
    or_d = nc.dram_tensor("or_out", [D, D], f16, kind="ExternalOutput").ap()
    oi_d = nc.dram_tensor("oi_out", [D, D], f16, kind="ExternalOutput").ap()
    # negated upper-right block of out_i; host transposes it into the
    # skipped lower-left block (out_i is antisymmetric)
    oin_d = nc.dram_tensor("oin_out", [D // 2, NW], f16, kind="ExternalOutput").ap()

    def ms(m):
        return slice(m * P, (m + 1) * P)

    def nsl(n):
        return slice(n * NW, (n + 1) * NW)

    with tile.TileContext(nc) as tc:
        with (
            tc.tile_pool(name="const", bufs=1) as cpool,
            tc.tile_pool(name="stage", bufs=1) as spool,
            tc.tile_pool(name="abc", bufs=1) as apool,
            tc.tile_pool(name="nsb", bufs=1) as npool,
            tc.tile_pool(name="osb", bufs=4) as opool,
            tc.tile_pool(name="ps", bufs=1, space="PSUM") as pspool,
        ):
            # Scales first on the sync HWDGE ring (the gpsimd SWDGE ring
            # takes ~1.5us to deliver its first bytes, which would gate
            # every scale-prep op and so the whole matmul stream).
            s_t = cpool.tile([P, KC], f32, name="s_t")
            nc.sync.dma_start(s_t[:], s_d)

            # PE prewarm on zeroed tiles: starts the p-state ramp during
            # the input-DMA head.
            zw = cpool.tile([P, 5 * P], f16, name="zw")
            nc.vector.memset(zw[:], 0.0)
            pz = pspool.tile([P, 4 * P], f32, name="pz", tag="ps0")
            for _ in range(N_PREWARM):
                nc.tensor.matmul(
                    pz[:], zw[:, 0:P], zw[:, P : 5 * P], start=True, stop=True
                )

            # Intake in [P, HD] half-units, k-ordered round-robin:
            # r halves alternate sync/scalar HWDGE rings, i halves stream
            # down the faster gpsimd SWDGE ring.  Separate tiles per k keep
            # the scale-prep dependencies tight.
            rt = [
                spool.tile([P, D], f16, name=f"r{k}", tag=f"r{k}")
                for k in range(KC)
            ]
            it = [
                spool.tile([P, D], f16, name=f"i{k}", tag=f"i{k}")
                for k in range(KC)
            ]
            # k-ordered half-unit round-robin, ring times balanced to the
            # measured ~90/97/112 GB/s (sync/scalar/gpsimd) rates; the
            # first-needed halves (r?a on sync, i?a on scalar) land in k
            # cadence, b-halves ride gpsimd and are only needed by the
            # second N group / AB prep.
            for k in range(KC):
                nc.sync.dma_start(rt[k][:, 0:HD], r_d[:, k * D : k * D + HD])
                nc.scalar.dma_start(it[k][:, 0:HD], i_d[:, k * D : k * D + HD])
            for k in range(KC):
                nc.gpsimd.dma_start(rt[k][:, HD:D], r_d[:, k * D + HD : (k + 1) * D])
            for k in range(KC - 1):
                nc.gpsimd.dma_start(it[k][:, HD:D], i_d[:, k * D + HD : (k + 1) * D])
            nc.sync.dma_start(it[KC - 1][:, HD:D], i_d[:, (KC - 1) * D + HD : KC * D])
            idn = cpool.tile([P, 3 * P], f16, name="idn")
            nc.gpsimd.dma_start(idn[:], e_d)
            pI = idn[:, 0:P]
            nI = idn[:, P : 2 * P]
            n2I = idn[:, 2 * P : 3 * P]

            # Per-row scaling on DVE: A = sw*R, B = sw*I (fp16), k=0 in
            # halves to cut first-matmul latency.  AB = A+B after all A/B.
            # All prep in halves on DVE, a-halves (N group a operands)
            # first in k order, b-halves after, then AB = A + B.
            At, Bt, ABt = [], [], []
            for k in range(KC):
                At.append(apool.tile([P, D], f16, name=f"A{k}", tag=f"A{k}"))
                Bt.append(apool.tile([P, D], f16, name=f"B{k}", tag=f"B{k}"))
            for k in range(KC):
                sk = s_t[:, k : k + 1]
                nc.vector.tensor_scalar_mul(At[k][:, 0:HD], rt[k][:, 0:HD], sk)
                nc.vector.tensor_scalar_mul(Bt[k][:, 0:HD], it[k][:, 0:HD], sk)
                nc.vector.tensor_scalar_mul(At[k][:, HD:D], rt[k][:, HD:D], sk)
            for k in range(KC):
                sk = s_t[:, k : k + 1]
                nc.vector.tensor_scalar_mul(Bt[k][:, HD:D], it[k][:, HD:D], sk)
            for k in range(KC):
                ab = apool.tile([P, D], f16, name=f"AB{k}", tag=f"AB{k}")
                nc.vector.tensor_tensor(ab[:], At[k][:], Bt[k][:], add_op)
                ABt.append(ab)

            # pN[m] = N row-block m, fp16 in SBUF (ScalarE evac copies).
            pn = [
                npool.tile([P, D], f16, name=f"pn{m}", tag=f"pn{m}")
                for m in range(MT)
            ]

            # N = B^T A, computed full.  Group a (rows 0-2) runs with k
            # outermost so PE chunk demand tracks DMA arrival; group b
            # (rows 3-5) runs k-inner per row -- by then every chunk has
            # landed, and per-row completion lets evacs (and the pair
            # phase) start sooner.  Evacs: h0 on ScalarE, h1 on DVE (idle
            # until the pair phase), so PSUM banks release promptly.
            def evac_row(m, psn_h0, psn_h1):
                nc.scalar.copy(pn[m][:, nsl(0)], psn_h0[:])
                nc.scalar.copy(pn[m][:, nsl(1)], psn_h1[:])

            psn_a = [
                pspool.tile([P, NW], f32, name=f"psna{j}", tag=f"ps{j}")
                for j in range(6)
            ]
            for k in range(KC):
                for mm in range(3):
                    for h in range(NB):
                        nc.tensor.matmul(
                            psn_a[2 * mm + h][:],
                            Bt[k][:, ms(mm)],
                            At[k][:, nsl(h)],
                            start=(k == 0),
                            stop=(k == KC - 1),
                        )
            for mm in range(3):
                evac_row(mm, psn_a[2 * mm], psn_a[2 * mm + 1])

            nb_tags = [("ps6", "ps7"), ("ps0", "ps1"), ("ps2", "ps3")]
            for mm in range(3):
                m = 3 + mm
                t0, t1 = nb_tags[mm]
                p0 = pspool.tile([P, NW], f32, name=f"psnb{m}0", tag=t0)
                p1 = pspool.tile([P, NW], f32, name=f"psnb{m}1", tag=t1)
                for k in range(KC):
                    nc.tensor.matmul(
                        p0[:], Bt[k][:, ms(m)], At[k][:, nsl(0)],
                        start=(k == 0), stop=(k == KC - 1),
                    )
                    nc.tensor.matmul(
                        p1[:], Bt[k][:, ms(m)], At[k][:, nsl(1)],
                        start=(k == 0), stop=(k == KC - 1),
                    )
                evac_row(m, p0, p1)

            # Pair phase.  Per (m,n) tile, in PE order:
            #   3 fp16 transposes of pN -> psnt (= N^T, fp16 bank)
            #   DVE: out_i = -psnt + pN    (fp16 store-staging tile)
            #   4 G matmuls + (-2I x pN) into psg
            #   (+I x out_i) into psg, closing it: psg = G - 2N + (N - N^T)
            #                                          = G - N - N^T = out_r
            #   [3 oin pairs: psoin = -I x out_i; ACT copy; store]
            #   DVE: out_r = cast(psg)   (emitted in the next pair's slot)
            def close_pair(j):
                """psg_j += out_i_j (closes the group), then cast+store.
                Issued during pair j+1 so the DVE oi pass has a full pair
                of slack before the PE consumes its result."""
                m, n = PAIRS[j]
                psg, oi_sb = psg_t[j], oi_t[j]
                nc.tensor.matmul(psg[:], pI, oi_sb[:], start=False, stop=True)
                if n == 1 and m < 3:
                    # oin = -out_i for the host antisym mirror
                    psoin = pspool.tile(
                        [P, NW], f32, name=f"psoin{j}", tag="ps6"
                    )
                    nc.tensor.matmul(psoin[:], nI, oi_sb[:], start=True, stop=True)
                    oin_sb = opool.tile(
                        [P, NW], f16, name=f"oin_sb{j}", tag="oin_sb"
                    )
                    nc.scalar.copy(oin_sb[:], psoin[:])
                    nc.scalar.dma_start(oin_d[ms(m), :], oin_sb[:])
                or_sb = opool.tile([P, NW], f16, name=f"or_sb{j}", tag="or_sb")
                nc.vector.tensor_copy(or_sb[:], psg[:])
                nc.sync.dma_start(or_d[ms(m), nsl(n)], or_sb[:])

            psg_t, oi_t = {}, {}
            for j, (m, n) in enumerate(PAIRS):
                psg = pspool.tile(
                    [P, NW], f32, name=f"psg{j}", tag=f"ps{(2 * j) % 6}"
                )
                psnt = pspool.tile(
                    [P, NW], f16, name=f"psnt{j}", tag=f"ps{(2 * j + 1) % 6}"
                )
                psg_t[j] = psg
                if j > 0:
                    close_pair(j - 1)
                for c in range(3):
                    cc = 3 * n + c
                    nc.tensor.matmul(
                        psnt[:, c * P : (c + 1) * P],
                        pn[cc][:, ms(m)],
                        pI,
                        is_transpose=True,
                        start=(c == 0),
                        stop=(c == 2),
                    )
                # out_i = -N^T + N
                oi_sb = opool.tile([P, NW], f16, name=f"oi_sb{j}", tag="oi_sb")
                nc.vector.scalar_tensor_tensor(
                    oi_sb[:], psnt[:], -1.0, pn[m][:, nsl(n)], mul_op, add_op
                )
                nc.gpsimd.dma_start(oi_d[ms(m), nsl(n)], oi_sb[:])
                oi_t[j] = oi_sb

                for k in range(KC):
                    nc.tensor.matmul(
                        psg[:], ABt[k][:, ms(m)], ABt[k][:, nsl(n)],
                        start=(k == 0), stop=False,
                    )
                # psg -= 2N
                nc.tensor.matmul(
                    psg[:], n2I, pn[m][:, nsl(n)], start=False, stop=False
                )
            close_pair(len(PAIRS) - 1)

    nc.compile()
    return nc


def get_nc():
    if "nc" not in _CACHE:
        _CACHE["nc"] = _build()
    return _CACHE["nc"]


def make_in_maps(input_real, input_imag, weight):
    input_real = np.asarray(input_real)
    input_imag = np.asarray(input_imag)
    weight = np.asarray(weight, dtype=np.float32)
    # pack [S, D] -> [P, KC*D]: row p holds chunks k=0..KC-1 concatenated
    r16 = (
        input_real.astype(np.float16)
        .reshape(B, KC, P, D)
        .transpose(0, 2, 1, 3)
        .reshape(B, P, KC * D)
    )
    i16 = (
        input_imag.astype(np.float16)
        .reshape(B, KC, P, D)
        .transpose(0, 2, 1, 3)
        .reshape(B, P, KC * D)
    )
    sq = np.sqrt(weight).astype(np.float32)  # [B, S]
    s_pack = sq.reshape(B, KC, P).transpose(0, 2, 1)  # [B, P, KC]
    eye = np.eye(P, dtype=np.float16)
    eye2 = np.ascontiguousarray(np.concatenate([eye, -eye, -2 * eye], axis=1))
    return [
        {
            "r_in": np.ascontiguousarray(r16[b]),
            "i_in": np.ascontiguousarray(i16[b]),
            "s_in": np.ascontiguousarray(s_pack[b]),
            "e_in": eye2,
        }
        for b in range(B)
    ]


def unshard_one(res: dict) -> tuple[np.ndarray, np.ndarray]:
    """Device outputs (fp16, upper tiles) -> full fp32 (out_r, out_i)."""
    out_r = np.asarray(res["or_out"]).astype(np.float32)
    out_i = np.asarray(res["oi_out"]).astype(np.float32)
    # Mirror the device-skipped strictly-lower blocks (pure transpose
    # copies): out_r is symmetric; out_i's mirror block was negated on
    # device into oin_out.
    out_r[NW:D, 0:NW] = out_r[0:NW, NW:D].T
    out_i[NW:D, 0:NW] = np.asarray(res["oin_out"]).astype(np.float32).T
    return out_r, out_i


def run(input_real, input_imag, weight, **spmd_kwargs):
    nc = get_nc()
    res = bass_utils.run_bass_kernel_spmd(
        nc,
        make_in_maps(input_real, input_imag, weight),
        core_ids=list(range(N_CORES)),
        **spmd_kwargs,
    )
    outs = [unshard_one(res.results[b]) for b in range(B)]
    out_r = np.stack([o[0] for o in outs])
    out_i = np.stack([o[1] for o in outs])
    return (out_r, out_i), res


def kernel(input_real, input_imag, weight):
    (out_r, out_i), _ = run(input_real, input_imag, weight)
    return (out_r, out_i)


# revision 18
# speedup vs baseline: 1.0074x; 1.0074x over previous
"""Trainium2 Bass kernel for nn_ComplexMixture.

Per batch element b (R = input_real[b] [S,D], I = input_imag[b] [S,D], w [S]):
    out_r = (w*R)^T R + (w*I)^T I        (symmetric)
    out_i = (w*I)^T R - (w*R)^T I        (antisymmetric)

With A = sqrt(w)*R, B = sqrt(w)*I (w >= 0):
    N  = B^T A            (full 768x768, 48 matmuls)
    G  = (A+B)^T (A+B)    (upper 9 of 12 [128,384] pair-tiles, 36 matmuls)
    out_r = G - N - N^T   (symmetric   -> upper tiles only)
    out_i = N - N^T       (antisymmetric -> upper tiles only)

vs the 4-term scheme (144 matmuls = 55.3k PE cycles) this needs 44.9k PE
cycles, and keeps single-pass PSUM evacuation: every out_r combine happens
ON THE PE inside the psG accumulation group
    psg = G(4 mm) - N(1 mm: lhsT=-I, rhs=pN) - N^T(1 mm: lhsT=-I, rhs=nt)
where pN = fp16 SBUF copy of N and nt = fp16 SBUF copy of psnt = N^T
(3 fp16 PE transposes of pN, ~56ns each).  The transposes lead each pair
so the nt evacuation hides under that same pair's G matmuls; no
cross-pair dependency chains and a ~1.3us closing tail.  Evac:
out_r = one DVE cast, out_i = one DVE scalar_tensor_tensor (-psnt + pN),
oin (= -out_i, host's antisymmetric mirror) = 2-matmul mini-bank + copy.

Timing structure (measured): exec ~= end-of-last-work + ~4.2us fixed exit
(drain + tile barrier + NEFF semaphore sweep).  The binding constraint on
the front is DMA intake: ~1.57MB of fp16 input over three rings at
~90/70/145 GB/s; the N phase consumes a k-chunk every ~2us, so chunks are
loaded in [P,384] half-units round-robined across rings in k order
(r halves on sync+scalar HWDGE, i halves on the faster gpsimd SWDGE) to
keep arrival cadence ahead of PE demand.  Dummy matmuls bridge the PE
from the preamble into the stream (p-state ramp).

Outputs are stored fp16 (host upcasts: pure dtype cast).  out_r's
strictly-lower [384,384] block and out_i's (via the device-negated oin
tile) are host-mirrored by pure transpose copies.  Sharding:
data-parallel over batch, one element per NeuronCore (B == 8 == n_cores).
"""

import sys
import types

import numpy as np

# If the environment requests tracing (BASS_TRACE=1) but the image lacks
# antenv.axon_hooks, bass_utils would crash importing it; provide a no-op
# hook registry so tracing degrades gracefully instead.
try:
    import antenv.axon_hooks  # noqa: F401
except ImportError:
    _hooks = types.ModuleType("antenv.axon_hooks")
    _hooks._hook = None
    _hooks.set_axon_ntff_profile_hook = lambda h: setattr(_hooks, "_hook", h)
    _hooks.get_axon_ntff_profile_hook = lambda: _hooks._hook
    sys.modules["antenv.axon_hooks"] = _hooks

import concourse.bacc as bacc
import concourse.bass_utils as bass_utils
import concourse.mybir as mybir
import concourse.tile as tile

B, S, D = 8, 512, 768
P = 128          # SBUF/PSUM partitions; matmul contraction tile
KC = S // P      # 4 contraction chunks per operand
MT = D // P      # 6 output row tiles
NW = 384         # matmul moving free dim (<=512 fp32 PSUM bank)
NB = D // NW     # 2 output column blocks
HD = D // 2      # half row width (intake half-units)
N_CORES = 8
N_PREWARM = 10   # dummy 512-col matmuls bridging preamble -> first real mm;
                 # ~3.7us of continuous PE activity so the HAM clock grant
                 # (full 2.4GHz ~6us after sustained activity starts) lands
                 # as early as possible

# 9 computed (m, n) pair-tiles: full upper at [384] granularity; n=0 pairs
# first (their transposes only need pN rows 0..2 from the first N group)
PAIRS = [(0, 0), (1, 0), (2, 0), (0, 1), (1, 1), (2, 1), (3, 1), (4, 1), (5, 1)]

_CACHE: dict = {}


def _build():
    f32, f16 = mybir.dt.float32, mybir.dt.float16
    add_op = mybir.AluOpType.add
    mul_op = mybir.AluOpType.mult
    nc = bacc.Bacc(
        "TRN2", target_bir_lowering=False, debug=False, num_devices=N_CORES
    )
    # Host-packed partition-major: r_in[p, k*D:(k+1)*D] = R[k*P+p, :].
    r_d = nc.dram_tensor("r_in", [P, KC * D], f16, kind="ExternalInput").ap()
    i_d = nc.dram_tensor("i_in", [P, KC * D], f16, kind="ExternalInput").ap()
    s_d = nc.dram_tensor("s_in", [P, KC], f32, kind="ExternalInput").ap()
    # [+I | -I | -2I] fp16
    e_d = nc.dram_tensor("e_in", [P, 3 * P], f16, kind="ExternalInput").ap()
    or_d = nc.dram_tensor("or_out", [D, D], f16, kind="ExternalOutput").ap()
    oi_d = nc.dram_tensor("oi_out", [D, D], f16, kind="ExternalOutput").ap()
    # negated upper-right block of out_i; host transposes it into the
    # skipped lower-left block (out_i is antisymmetric)
    oin_d = nc.dram_tensor("oin_out", [D // 2, NW], f16, kind="ExternalOutput").ap()

    def ms(m):
        return slice(m * P, (m + 1) * P)

    def nsl(n):
        return slice(n * NW, (n + 1) * NW)

    with tile.TileContext(nc) as tc:
        with (
            tc.tile_pool(name="const", bufs=1) as cpool,
            tc.tile_pool(name="stage", bufs=1) as spool,
            tc.tile_pool(name="abc", bufs=1) as apool,
            tc.tile_pool(name="nsb", bufs=1) as npool,
            tc.tile_pool(name="osb", bufs=4) as opool,
            tc.tile_pool(name="ps", bufs=1, space="PSUM") as pspool,
        ):
            # Scales first on the sync HWDGE ring (the gpsimd SWDGE ring
            # takes ~1.5us to deliver its first bytes, which would gate
            # every scale-prep op and so the whole matmul stream).
            s_t = cpool.tile([P, KC], f32, name="s_t")
            nc.scalar.dma_start(s_t[:], s_d)

            # PE prewarm on zeroed tiles: starts the p-state ramp during
            # the input-DMA head.
            zw = cpool.tile([P, 5 * P], f16, name="zw")
            nc.vector.memset(zw[:], 0.0)
            pz = pspool.tile([P, 4 * P], f32, name="pz", tag="ps0")
            for _ in range(N_PREWARM):
                nc.tensor.matmul(
                    pz[:], zw[:, 0:P], zw[:, P : 5 * P], start=True, stop=True
                )

            # Intake in [P, HD] half-units, k-ordered round-robin:
            # r halves alternate sync/scalar HWDGE rings, i halves stream
            # down the faster gpsimd SWDGE ring.  Separate tiles per k keep
            # the scale-prep dependencies tight.
            rt = [
                spool.tile([P, D], f16, name=f"r{k}", tag=f"r{k}")
                for k in range(KC)
            ]
            it = [
                spool.tile([P, D], f16, name=f"i{k}", tag=f"i{k}")
                for k in range(KC)
            ]
            # k-ordered half-unit round-robin, ring times balanced to the
            # measured ~90/97/112 GB/s (sync/scalar/gpsimd) rates; the
            # first-needed halves (r?a on sync, i?a on scalar) land in k
            # cadence, b-halves ride gpsimd and are only needed by the
            # second N group / AB prep.
            for k in range(KC):
                nc.sync.dma_start(rt[k][:, 0:HD], r_d[:, k * D : k * D + HD])
                nc.scalar.dma_start(it[k][:, 0:HD], i_d[:, k * D : k * D + HD])
            for k in range(KC):
                nc.gpsimd.dma_start(rt[k][:, HD:D], r_d[:, k * D + HD : (k + 1) * D])
            for k in range(KC - 1):
                nc.gpsimd.dma_start(it[k][:, HD:D], i_d[:, k * D + HD : (k + 1) * D])
            nc.sync.dma_start(it[KC - 1][:, HD:D], i_d[:, (KC - 1) * D + HD : KC * D])
            idn = cpool.tile([P, 3 * P], f16, name="idn")
            nc.gpsimd.dma_start(idn[:], e_d)
            pI = idn[:, 0:P]
            nI = idn[:, P : 2 * P]
            n2I = idn[:, 2 * P : 3 * P]

            # Per-row scaling on DVE: A = sw*R, B = sw*I (fp16), k=0 in
            # halves to cut first-matmul latency.  AB = A+B after all A/B.
            # All prep in halves on DVE, a-halves (N group a operands)
            # first in k order, b-halves after, then AB = A + B.
            At, Bt, ABt = [], [], []
            for k in range(KC):
                At.append(apool.tile([P, D], f16, name=f"A{k}", tag=f"A{k}"))
                Bt.append(apool.tile([P, D], f16, name=f"B{k}", tag=f"B{k}"))
            for k in range(KC):
                sk = s_t[:, k : k + 1]
                nc.vector.tensor_scalar_mul(At[k][:, 0:HD], rt[k][:, 0:HD], sk)
                nc.vector.tensor_scalar_mul(Bt[k][:, 0:HD], it[k][:, 0:HD], sk)
                nc.vector.tensor_scalar_mul(At[k][:, HD:D], rt[k][:, HD:D], sk)
            for k in range(KC):
                sk = s_t[:, k : k + 1]
                nc.vector.tensor_scalar_mul(Bt[k][:, HD:D], it[k][:, HD:D], sk)
            for k in range(KC):
                ab = apool.tile([P, D], f16, name=f"AB{k}", tag=f"AB{k}")
                nc.vector.tensor_tensor(ab[:], At[k][:], Bt[k][:], add_op)
                ABt.append(ab)

            # pN[m] = N row-block m, fp16 in SBUF (ScalarE evac copies).
            pn = [
                npool.tile([P, D], f16, name=f"pn{m}", tag=f"pn{m}")
                for m in range(MT)
            ]

            # N = B^T A, computed full.  Group a (rows 0-2) runs with k
            # outermost so PE chunk demand tracks DMA arrival; group b
            # (rows 3-5) runs k-inner per row -- by then every chunk has
            # landed, and per-row completion lets evacs (and the pair
            # phase) start sooner.  Evacs: h0 on ScalarE, h1 on DVE (idle
            # until the pair phase), so PSUM banks release promptly.
            def evac_row(m, psn_h0, psn_h1):
                nc.scalar.copy(pn[m][:, nsl(0)], psn_h0[:])
                nc.scalar.copy(pn[m][:, nsl(1)], psn_h1[:])

            psn_a = [
                pspool.tile([P, NW], f32, name=f"psna{j}", tag=f"ps{j}")
                for j in range(6)
            ]
            for k in range(KC):
                for mm in range(3):
                    for h in range(NB):
                        nc.tensor.matmul(
                            psn_a[2 * mm + h][:],
                            Bt[k][:, ms(mm)],
                            At[k][:, nsl(h)],
                            start=(k == 0),
                            stop=(k == KC - 1),
                        )
            for mm in range(3):
                evac_row(mm, psn_a[2 * mm], psn_a[2 * mm + 1])

            nb_tags = [("ps6", "ps7"), ("ps0", "ps1"), ("ps2", "ps3")]
            for mm in range(3):
                m = 3 + mm
                t0, t1 = nb_tags[mm]
                p0 = pspool.tile([P, NW], f32, name=f"psnb{m}0", tag=t0)
                p1 = pspool.tile([P, NW], f32, name=f"psnb{m}1", tag=t1)
                for k in range(KC):
                    nc.tensor.matmul(
                        p0[:], Bt[k][:, ms(m)], At[k][:, nsl(0)],
                        start=(k == 0), stop=(k == KC - 1),
                    )
                    nc.tensor.matmul(
                        p1[:], Bt[k][:, ms(m)], At[k][:, nsl(1)],
                        start=(k == 0), stop=(k == KC - 1),
                    )
                evac_row(m, p0, p1)

            # Pair phase.  Per (m,n) tile, in PE order:
            #   3 fp16 transposes of pN -> psnt (= N^T, fp16 bank)
            #   DVE: out_i = -psnt + pN    (fp16 store-staging tile)
            #   4 G matmuls + (-2I x pN) into psg
            #   (+I x out_i) into psg, closing it: psg = G - 2N + (N - N^T)
            #                                          = G - N - N^T = out_r
            #   [3 oin pairs: psoin = -I x out_i; ACT copy; store]
            #   DVE: out_r = cast(psg)   (emitted in the next pair's slot)
            def close_pair(j):
                """psg_j += out_i_j (closes the group), then cast+store.
                Issued during pair j+1 so the DVE oi pass has a full pair
                of slack before the PE consumes its result."""
                m, n = PAIRS[j]
                psg, oi_sb = psg_t[j], oi_t[j]
                nc.tensor.matmul(psg[:], pI, oi_sb[:], start=False, stop=True)
                if n == 1 and m < 3:
                    # oin = -out_i for the host antisym mirror
                    psoin = pspool.tile(
                        [P, NW], f32, name=f"psoin{j}", tag="ps6"
                    )
                    nc.tensor.matmul(psoin[:], nI, oi_sb[:], start=True, stop=True)
                    oin_sb = opool.tile(
                        [P, NW], f16, name=f"oin_sb{j}", tag="oin_sb"
                    )
                    nc.scalar.copy(oin_sb[:], psoin[:])
                    nc.scalar.dma_start(oin_d[ms(m), :], oin_sb[:])
                or_sb = opool.tile([P, NW], f16, name=f"or_sb{j}", tag="or_sb")
                nc.vector.tensor_copy(or_sb[:], psg[:])
                nc.sync.dma_start(or_d[ms(m), nsl(n)], or_sb[:])

            psg_t, oi_t = {}, {}
            for j, (m, n) in enumerate(PAIRS):
                psg = pspool.tile(
                    [P, NW], f32, name=f"psg{j}", tag=f"ps{(2 * j) % 6}"
                )
                psnt = pspool.tile(
                    [P, NW], f16, name=f"psnt{j}", tag=f"ps{(2 * j + 1) % 6}"
                )
                psg_t[j] = psg
                if j > 0:
                    close_pair(j - 1)
                for c in range(3):
                    cc = 3 * n + c
                    nc.tensor.matmul(
                        psnt[:, c * P : (c + 1) * P],
                        pn[cc][:, ms(m)],
                        pI,
                        is_transpose=True,
                        start=(c == 0),
                        stop=(c == 2),
                    )
                # out_i = -N^T + N
                oi_sb = opool.tile([P, NW], f16, name=f"oi_sb{j}", tag="oi_sb")
                nc.vector.scalar_tensor_tensor(
                    oi_sb[:], psnt[:], -1.0, pn[m][:, nsl(n)], mul_op, add_op
                )
                nc.gpsimd.dma_start(oi_d[ms(m), nsl(n)], oi_sb[:])
                oi_t[j] = oi_sb

                for k in range(KC):
                    nc.tensor.matmul(
                        psg[:], ABt[k][:, ms(m)], ABt[k][:, nsl(n)],
                        start=(k == 0), stop=False,
                    )
                # psg -= 2N
                nc.tensor.matmul(
                    psg[:], n2I, pn[m][:, nsl(n)], start=False, stop=False
                )
            close_pair(len(PAIRS) - 1)

    nc.compile()
    return nc


def get_nc():
    if "nc" not in _CACHE:
        _CACHE["nc"] = _build()
    return _CACHE["nc"]


def make_in_maps(input_real, input_imag, weight):
    input_real = np.asarray(input_real)
    input_imag = np.asarray(input_imag)
    weight = np.asarray(weight, dtype=np.float32)
    # pack [S, D] -> [P, KC*D]: row p holds chunks k=0..KC-1 concatenated
    r16 = (
        input_real.astype(np.float16)
        .reshape(B, KC, P, D)
        .transpose(0, 2, 1, 3)
        .reshape(B, P, KC * D)
    )
    i16 = (
        input_imag.astype(np.float16)
        .reshape(B, KC, P, D)
        .transpose(0, 2, 1, 3)
        .reshape(B, P, KC * D)
    )
    sq = np.sqrt(weight).astype(np.float32)  # [B, S]
    s_pack = sq.reshape(B, KC, P).transpose(0, 2, 1)  # [B, P, KC]
    eye = np.eye(P, dtype=np.float16)
    eye2 = np.ascontiguousarray(np.concatenate([eye, -eye, -2 * eye], axis=1))
    return [
        {
            "r_in": np.ascontiguousarray(r16[b]),
            "i_in": np.ascontiguousarray(i16[b]),
            "s_in": np.ascontiguousarray(s_pack[b]),
            "e_in": eye2,
        }
        for b in range(B)
    ]


def unshard_one(res: dict) -> tuple[np.ndarray, np.ndarray]:
    """Device outputs (fp16, upper tiles) -> full fp32 (out_r, out_i)."""
    out_r = np.asarray(res["or_out"]).astype(np.float32)
    out_i = np.asarray(res["oi_out"]).astype(np.float32)
    # Mirror the device-skipped strictly-lower blocks (pure transpose
    # copies): out_r is symmetric; out_i's mirror block was negated on
    # device into oin_out.
    out_r[NW:D, 0:NW] = out_r[0:NW, NW:D].T
    out_i[NW:D, 0:NW] = np.asarray(res["oin_out"]).astype(np.float32).T
    return out_r, out_i


def run(input_real, input_imag, weight, **spmd_kwargs):
    nc = get_nc()
    res = bass_utils.run_bass_kernel_spmd(
        nc,
        make_in_maps(input_real, input_imag, weight),
        core_ids=list(range(N_CORES)),
        **spmd_kwargs,
    )
    outs = [unshard_one(res.results[b]) for b in range(B)]
    out_r = np.stack([o[0] for o in outs])
    out_i = np.stack([o[1] for o in outs])
    return (out_r, out_i), res


def kernel(input_real, input_imag, weight):
    (out_r, out_i), _ = run(input_real, input_imag, weight)
    return (out_r, out_i)
